# revision 24
# baseline (speedup 1.0000x reference)
"""Trainium2 Bass kernel for a 2-layer LSTM (B=2048, T=512, IN=2, H=64) + FC head.

Data-parallel over 8 NeuronCores: batch 2048 -> 256 per core, weights replicated.

On-chip layout (per core, B_local = 256):
  - Hidden/cell state of BOTH layers packed in one [128, 256] SBUF tile:
    rows 0:64 = layer0 h (or c), rows 64:128 = layer1 h (or c); free dim = batch.
  - Software pipeline: iteration i computes layer0 @ t=i and layer1 @ t=i-1.
  - PSUM: one bank (512 f32) per gate PAIR, two tiles per step: pfi = [f|i],
    pgo = [g|o], 4-deep rotation (8 banks total). Per-pair tiles make the
    Tile dependency tracker release the f,i sigmoid after only the f,i
    recurrent matmuls; the 4-deep rotation lets x-inject matmuls for step
    t+4 run during step t's activation tail (keeps the PE dense and off the
    critical path).
  - Per step the PE runs [x-inject(t+1) K=3 MMs x4] then [recurrent K=128
    MMs x4]; x-injects are emitted first (no h(t) dependency).
  - Biases ride the ones-row of the x-inject matmul (w0xb).
  - VectorE: c = f*c + i*g ; h = o * tanh(c)  (tanh/sigmoid on ScalarE).
  - Matmul operands are bf16; PSUM fp32; activations/cell state bf16.

Measured on trn2 (512 steps): 1.686 ms (3294 ns/step; baseline 1.979 ms).
The period is bound by the serial per-step chain
  rec-MM(f,i) 585 -> sigmoid(f|i) 687 -> tanh(g) 363 -> i*g 292 -> c-add 292
  -> tanh(c) 507 -> h-mul 292 (+ ~290 semaphore gaps),
with ScalarE ~63% busy. Explored and rejected (all lose to the ~260-360ns
fixed per-instruction overheads of ScalarE/VectorE at FD=128-256, or to
head-of-line blocking in the engines' FIFO queues): 2-way batch-split
software pipelining, tanh-as-sigmoid gate folding, tensor_tensor_scan
fusion of the c-update, batch-on-partition transposed layouts, splitting
the tanh(c)/h tail, GPSIMD offload, and HAM warm-up/filler matmuls (PE runs
cold at 1.2 GHz; warming it is worth ~240ns/step but fillers block the
recurrent MMs and HAM re-throttles regardless).
"""

import numpy as np
from contextlib import contextmanager

B, T, IN, H, NCLS = 2048, 512, 2, 64, 9
NCORES = 8
BL = B // NCORES          # 256 batch per core
TC = 16                   # timesteps per x chunk DMA

import os as _os

MM_BF16 = _os.environ.get("MM_BF16", "1") == "1"    # bf16 matmul operands
ACT_BF16 = _os.environ.get("ACT_BF16", "1") == "1"  # bf16 gate activations
C_BF16 = _os.environ.get("C_BF16", "1") == "1"      # bf16 cell state
XR = 3  # x rows: x0, x1, ones

LAST_EXEC_NS = None
LAST_TRACE = None

_CACHE = {}


def _np_mmdt():
    if MM_BF16:
        import ml_dtypes
        return ml_dtypes.bfloat16
    return np.float32


def _build(steps=T):
    import concourse.bacc as bacc
    import concourse.tile as tile
    from concourse import mybir
    from contextlib import ExitStack

    f32 = mybir.dt.float32
    mdt = mybir.dt.bfloat16 if MM_BF16 else f32
    adt = mybir.dt.bfloat16 if ACT_BF16 else f32
    cdt = mybir.dt.bfloat16 if C_BF16 else f32
    AF = mybir.ActivationFunctionType
    SIG, TANH = AF.Sigmoid, AF.Tanh

    nc = bacc.Bacc(None, target_bir_lowering=False)

    x_d = nc.dram_tensor("x", [XR, steps * BL], mdt, kind="ExternalInput")
    w1_d = nc.dram_tensor("w1", [2 * H, 4 * H], mdt, kind="ExternalInput")   # [w_ih1;w_hh1].T
    wc_d = nc.dram_tensor("wc", [2 * H, 8 * H], mdt, kind="ExternalInput")   # fused per-gate [128,128]
    w0xb_d = nc.dram_tensor("w0xb", [XR, 8 * H], mdt, kind="ExternalInput")  # x-inject+bias per-gate [3,128]
    bias_d = nc.dram_tensor("bias", [2 * H, 4], f32, kind="ExternalInput")
    wfc_d = nc.dram_tensor("wfc", [2 * H, NCLS], mdt, kind="ExternalInput")  # top half zeros
    bfc_d = nc.dram_tensor("bfc", [NCLS, 1], f32, kind="ExternalInput")
    out_d = nc.dram_tensor("out", [NCLS, BL], f32, kind="ExternalOutput")

    with tile.TileContext(nc) as tc, ExitStack() as ctx:
        consts = ctx.enter_context(tc.tile_pool(name="consts", bufs=1))
        state = ctx.enter_context(tc.tile_pool(name="state", bufs=1))
        xpool = ctx.enter_context(tc.tile_pool(name="xchunk", bufs=2))
        acts = ctx.enter_context(tc.tile_pool(name="acts", bufs=2))
        NWARM = int(_os.environ.get("WARMUP", "0"))
        NFILL = int(_os.environ.get("FILLERS", "0"))
        # 8 PSUM banks: pfi + pgo tags, [128, 512] (1 bank) each, 4-deep
        # rotation -> x-injects for step t+4 can run as soon as step t's
        # activations have read their banks. (3-deep + 1 scratch bank when
        # the HAM warmup/filler experiment is enabled.)
        psum = ctx.enter_context(tc.tile_pool(
            name="psum", bufs=(3 if (NWARM or NFILL) else 4), space="PSUM"))

        def load_const(shape, dt, dram, tag):
            t = consts.tile(shape, dt, tag=tag)
            nc.sync.dma_start(t[:], dram[:, :])
            return t

        # Load order matters: the Sync DMA queue serializes at ~1.4us per
        # transfer, so fetch what the first compute needs (x chunk, w0xb for
        # the x-injects, wc for the recurrent MMs) before the tail consts
        # used only later.
        tc0 = min(TC, steps)
        xch0 = xpool.tile([XR, TC * BL], mdt)
        nc.sync.dma_start(xch0[:, : tc0 * BL], x_d[:, 0 : tc0 * BL])
        w0xb = load_const([XR, 8 * H], mdt, w0xb_d, "w0xb")
        wc = load_const([2 * H, 8 * H], mdt, wc_d, "wc")
        w1 = load_const([2 * H, 4 * H], mdt, w1_d, "w1")
        bias = load_const([2 * H, 4], f32, bias_d, "bias")
        wfc = load_const([2 * H, NCLS], mdt, wfc_d, "wfc")
        bfc = load_const([NCLS, 1], f32, bfc_d, "bfc")

        # h state is split into batch-half tiles so the recurrent matmuls of
        # half a can start after only HMUL_a (separate tiles = no false
        # tile-granular dependency on HMUL_b). c stays one tile (read/written
        # full-width).
        HB = BL // 2
        h_a = state.tile([2 * H, HB], mdt)
        h_b = state.tile([2 * H, HB], mdt)
        c_all = state.tile([2 * H, BL], cdt)
        nc.vector.memset(h_a[:], 0.0)
        nc.vector.memset(h_b[:], 0.0)
        nc.vector.memset(c_all[:], 0.0)
        hs = (h_a, h_b)

        # gate order in the 4H weight dim (PyTorch): i, f, g, o
        GI, GF, GG, GO = 0, 1, 2, 3
        gsl = lambda j: slice(j * H, (j + 1) * H)
        BW = 512
        # One PSUM bank (512 f32) per gate PAIR: pfi = [f | i], pgo = [g | o],
        # each [128, 512] with the two gates at cols 0:256 / 256:512. The
        # pair's first x-inject uses start=True (clears the whole bank); the
        # second uses start=False, which OVERWRITES its freshly-cleared region
        # (has_written=0) rather than accumulating. Recurrent MMs then
        # accumulate (has_written=1). Contiguous pairs make SIGfi a plain 2D
        # FD=512 read, and 1-bank tiles allow bufs=4 (8 banks total) for
        # deeper x-inject lookahead.
        GLOC = {GF: (0, 0, True), GI: (0, BL, False),
                GG: (1, 0, True), GO: (1, BL, False)}

        xch = None
        cur = None          # psum tiles (pfi, pgo) for the current step
        nxt = None          # psum tiles pre-written with x-inject for step t+1

        # HAM warm-up + per-step fillers (experiment, off by default): a ~3us
        # dense MM burst flips the PE clock gate 1.2 -> 2.4 GHz (measured:
        # spacing 107 -> 56ns after ~25 MMs) and fillers try to keep it warm.
        # Measured net-NEGATIVE: the fillers head-of-line-block the recurrent
        # MMs in the PE FIFO (+120ns/step) and HAM re-throttles anyway.
        scratch = None
        if NWARM or NFILL:
            scratch = psum.tile([2 * H, BW], f32, tag="scratch", bufs=1)
        for _ in range(NWARM):
            nc.tensor.matmul(scratch[:, 0:128], lhsT=wc[:, 0:128],
                             rhs=wc[:, 0:128], start=True, stop=True,
                             skip_group_check=True)

        def xinj(t, xt):
            """Allocate psum tiles for step t and run the 4 x-inject MMs."""
            pfi = psum.tile([2 * H, BW], f32, tag="pfi")
            pgo = psum.tile([2 * H, BW], f32, tag="pgo")
            tiles = (pfi, pgo)
            for j in (GF, GI, GG, GO):
                ti, co, st = GLOC[j]
                nc.tensor.matmul(
                    tiles[ti][:, co : co + BL],
                    lhsT=w0xb[:, 2 * H * j : 2 * H * (j + 1)],
                    rhs=xt, start=st, stop=False, skip_group_check=True,
                )
            return tiles

        def xt_of(t):
            return xch[:, (t % TC) * BL : (t % TC + 1) * BL]

        for it in range(steps + 1):
            do0 = it < steps

            if it == 0:
                xch = xch0
                nxt = xinj(0, xt_of(0))

            cur, nxt = nxt, None

            if do0:
                # current step's gate matmuls first need h(t); meanwhile the
                # next step's x-injects (below, issued first in FIFO) run.
                if it + 1 < steps:
                    if (it + 1) % TC == 0:
                        tc_nx = min(TC, steps - (it + 1))
                        xch = xpool.tile([XR, TC * BL], mdt)
                        nc.sync.dma_start(
                            xch[:, : tc_nx * BL],
                            x_d[:, (it + 1) * BL : (it + 1 + tc_nx) * BL],
                        )
                    nxt = xinj(it + 1, xt_of(it + 1))
                # recurrent + layer1-inject: one K=128 MM per (gate, batch
                # half); half-a MMs depend only on h_a so they start while
                # HMUL_b is still writing h_b. f,i first (SIGfi fires after
                # the 4th), then g (TANHg), then o.
                for j, hf in ((GF, 0), (GI, 0), (GF, 1), (GI, 1),
                              (GG, 0), (GG, 1), (GO, 0), (GO, 1)):
                    ti, co, _ = GLOC[j]
                    nc.tensor.matmul(
                        cur[ti][:, co + HB * hf : co + HB * (hf + 1)],
                        lhsT=wc[:, 2 * H * j : 2 * H * (j + 1)],
                        rhs=hs[hf][:, :], start=False, stop=True,
                        skip_group_check=True,
                    )
                # N=64 fillers: ~55ns each (NX dispatch floor), so they keep
                # the PE busy for HAM without coarse-grained blocking of the
                # next recurrent burst in the PE FIFO.
                for _ in range(NFILL):
                    nc.tensor.matmul(scratch[:, 0:64], lhsT=wc[:, 0:128],
                                     rhs=wc[:, 0:64], start=True, stop=True,
                                     skip_group_check=True)
                lo, hi = 0, (2 * H if it >= 1 else H)
            else:
                # final iteration: layer1 only @ t = steps-1
                pfi = psum.tile([2 * H, BW], f32, tag="pfi")
                pgo = psum.tile([2 * H, BW], f32, tag="pgo")
                cur = (pfi, pgo)
                for j, hf in ((GF, 0), (GI, 0), (GF, 1), (GI, 1),
                              (GG, 0), (GG, 1), (GO, 0), (GO, 1)):
                    ti, co, st = GLOC[j]
                    nc.tensor.matmul(
                        cur[ti][H : 2 * H, co + HB * hf : co + HB * (hf + 1)],
                        lhsT=w1[:, gsl(j)], rhs=hs[hf][:, :],
                        start=(st and hf == 0), stop=True,
                        skip_group_check=True,
                    )
                lo, hi = H, 2 * H

            sl = slice(lo, hi)
            bias_kw = {}
            if not do0:
                # biases normally ride the x-inject ones-row; the final
                # L1-only step has no x-inject, so use the ACT bias operand.
                bias_kw = {GF: dict(bias=bias[sl, GF : GF + 1]),
                           GI: dict(bias=bias[sl, GI : GI + 1]),
                           GG: dict(bias=bias[sl, GG : GG + 1]),
                           GO: dict(bias=bias[sl, GO : GO + 1])}

            s_fi = acts.tile([2 * H, 2 * BL], adt, tag="sfi")
            if do0:
                nc.scalar.activation(s_fi[sl, :], cur[0][sl, :], SIG)
            else:
                nc.scalar.activation(s_fi[sl, 0:BL], cur[0][sl, 0:BL], SIG,
                                     **bias_kw[GF])
                nc.scalar.activation(s_fi[sl, BL : 2 * BL], cur[0][sl, BL : 2 * BL],
                                     SIG, **bias_kw[GI])
            s_g = acts.tile([2 * H, BL], adt, tag="sg")
            nc.scalar.activation(s_g[sl, :], cur[1][sl, 0:BL], TANH,
                                 **(bias_kw.get(GG, {}) if not do0 else {}))
            s_o = acts.tile([2 * H, BL], adt, tag="so")
            nc.scalar.activation(s_o[sl, :], cur[1][sl, BL : 2 * BL], SIG,
                                 **(bias_kw.get(GO, {}) if not do0 else {}))
            f_ap, i_ap = s_fi[:, 0:BL], s_fi[:, BL : 2 * BL]

            t_fc = acts.tile([2 * H, BL], cdt, tag="tfc")
            nc.vector.tensor_mul(t_fc[sl, :], f_ap[sl, :], c_all[sl, :])
            t_ig = acts.tile([2 * H, BL], cdt, tag="tig")
            nc.vector.tensor_mul(t_ig[sl, :], i_ap[sl, :], s_g[sl, :])
            nc.vector.tensor_add(c_all[sl, :], t_fc[sl, :], t_ig[sl, :])
            s_tc = acts.tile([2 * H, BL], adt, tag="stc")
            nc.scalar.activation(s_tc[sl, :], c_all[sl, :], TANH)
            nc.vector.tensor_mul(h_a[sl, :], s_o[sl, 0:HB], s_tc[sl, 0:HB])
            nc.vector.tensor_mul(h_b[sl, :], s_o[sl, HB:BL], s_tc[sl, HB:BL])

        # FC head on h2(T-1) = rows 64:128 of h; wfc is zero-padded on rows 0:64
        p_fc = psum.tile([2 * H, BW], f32, tag="pfi")
        nc.tensor.matmul(p_fc[0:NCLS, 0:HB], lhsT=wfc[:, :], rhs=h_a[:, :],
                         start=True, stop=False)
        nc.tensor.matmul(p_fc[0:NCLS, HB:BL], lhsT=wfc[:, :], rhs=h_b[:, :],
                         start=False, stop=True)
        o_sb = acts.tile([2 * H, BL], f32, tag="osb")
        nc.vector.tensor_scalar_add(o_sb[0:NCLS, :], p_fc[0:NCLS, 0:BL], bfc[:, 0:1])
        nc.sync.dma_start(out_d[:, :], o_sb[0:NCLS, :])

    nc.finalize()
    return nc


def _prep_weights(w_ih0, w_hh0, b_ih0, b_hh0, w_ih1, w_hh1, b_ih1, b_hh1, w_fc, b_fc):
    f = np.float32
    mdt = _np_mmdt()
    w1 = np.ascontiguousarray(
        np.concatenate([np.asarray(w_ih1), np.asarray(w_hh1)], 1).T
    ).astype(mdt)                                                        # [128, 256]
    # fused per-gate [K=128, M=128] blocks: cols 0:64 -> layer0 gate (zeros on
    # h2 rows), cols 64:128 -> layer1 gate ([w_ih1; w_hh1])
    wcf = np.zeros((2 * H, 8 * H), dtype=np.float32)
    w1f = np.concatenate([np.asarray(w_ih1), np.asarray(w_hh1)], 1)  # [256, 128]
    for g in range(4):
        wcf[0:H, 2 * H * g : 2 * H * g + H] = np.asarray(w_hh0)[g * H:(g + 1) * H, :].T
        wcf[:, 2 * H * g + H : 2 * H * (g + 1)] = w1f[g * H:(g + 1) * H, :].T
    wc = np.ascontiguousarray(wcf).astype(mdt)
    b0v = (np.asarray(b_ih0) + np.asarray(b_hh0)).astype(np.float32)
    b1v = (np.asarray(b_ih1) + np.asarray(b_hh1)).astype(np.float32)
    w0xbf = np.zeros((XR, 8 * H), dtype=np.float32)
    for g in range(4):
        w0xbf[0:IN, 2 * H * g : 2 * H * g + H] = \
            np.asarray(w_ih0)[g * H:(g + 1) * H, :].T
        w0xbf[IN, 2 * H * g : 2 * H * g + H] = b0v[g * H:(g + 1) * H]
        w0xbf[IN, 2 * H * g + H : 2 * H * (g + 1)] = b1v[g * H:(g + 1) * H]
    w0xb = np.ascontiguousarray(w0xbf).astype(mdt)
    b0 = (np.asarray(b_ih0) + np.asarray(b_hh0)).astype(f).reshape(4, H)
    b1 = (np.asarray(b_ih1) + np.asarray(b_hh1)).astype(f).reshape(4, H)
    bias = np.ascontiguousarray(np.concatenate([b0.T, b1.T], axis=0), dtype=f)
    wfc = np.zeros((2 * H, NCLS), dtype=f)
    wfc[H:, :] = np.asarray(w_fc).T
    wfc = wfc.astype(mdt)
    bfc = np.ascontiguousarray(np.asarray(b_fc).reshape(NCLS, 1), dtype=f)
    return dict(w1=w1, wc=wc, w0xb=w0xb, bias=bias, wfc=wfc, bfc=bfc)


def _prep_x(x, steps=T):
    mdt = _np_mmdt()
    x = np.asarray(x, dtype=np.float32)
    per_core = []
    for c in range(NCORES):
        xc = x[c * BL : (c + 1) * BL, :steps, :]          # [BL, steps, IN]
        xc = xc.transpose(2, 1, 0).reshape(IN, steps * BL)  # [IN, steps*BL]
        xa = np.ones((XR, steps * BL), dtype=np.float32)
        xa[0:IN] = xc
        per_core.append(np.ascontiguousarray(xa).astype(mdt))
    return per_core


@contextmanager
def _fast_compile():
    """Disable walrus birsim (compile-time BIR simulation): it costs ~7s per
    LSTM step (~1h for T=512) and only re-verifies what CoreSim already
    checked. NEFF output is identical."""
    import concourse.bass_utils as bu

    orig = bu.run_command

    def patched(argv, **kw):
        argv = [
            a.replace("--enable-birsim=true", "--enable-birsim=false")
            if isinstance(a, str) else a
            for a in argv
        ]
        return orig(argv, **kw)

    bu.run_command = patched
    try:
        yield
    finally:
        bu.run_command = orig


def kernel(x, w_ih0, w_hh0, b_ih0, b_hh0, w_ih1, w_hh1, b_ih1, b_hh1,
           w_fc, b_fc, _steps=T, _trace=False):
    global LAST_EXEC_NS, LAST_TRACE
    from concourse.bass_utils import run_bass_kernel_spmd

    key = ("nc", _steps)
    if key not in _CACHE:
        _CACHE[key] = _build(steps=_steps)
    nc = _CACHE[key]

    wmap = _prep_weights(w_ih0, w_hh0, b_ih0, b_hh0,
                         w_ih1, w_hh1, b_ih1, b_hh1, w_fc, b_fc)
    xs = _prep_x(x, _steps)
    in_maps = [{"x": xs[c], **wmap} for c in range(NCORES)]

    with _fast_compile():
        res = run_bass_kernel_spmd(nc, in_maps, core_ids=list(range(NCORES)),
                                   trace=_trace)
    LAST_EXEC_NS = res.exec_time_ns
    LAST_TRACE = res.instructions_and_trace
    out = np.concatenate([r["out"].T for r in res.results], axis=0)  # [B, 9]
    return out.astype(np.float32)


# revision 37
# speedup vs baseline: 1.0322x; 1.0322x over previous
"""Trainium2 Bass kernel for a 2-layer LSTM (B=2048, T=512, IN=2, H=64) + FC head.

Data-parallel over 8 NeuronCores: batch 2048 -> 256 per core, weights replicated.

On-chip layout (per core, B_local = 256):
  - Hidden/cell state of BOTH layers packed in one [128, 256] SBUF tile:
    rows 0:64 = layer0 h (or c), rows 64:128 = layer1 h (or c); free dim = batch.
  - Software pipeline: iteration i computes layer0 @ t=i and layer1 @ t=i-1.
  - PSUM: one bank (512 f32) per gate PAIR, two tiles per step: pfi = [f|i],
    pgo = [g|o], 4-deep rotation (8 banks total). Per-pair tiles make the
    Tile dependency tracker release the f,i sigmoid after only the f,i
    recurrent matmuls; the 4-deep rotation lets x-inject matmuls for step
    t+4 run during step t's activation tail (keeps the PE dense and off the
    critical path).
  - Per step the PE runs [x-inject(t+1) K=3 MMs x4] then [recurrent K=128
    MMs x4]; x-injects are emitted first (no h(t) dependency).
  - Biases ride the ones-row of the x-inject matmul (w0xb).
  - VectorE: c = f*c + i*g ; h = o * tanh(c)  (tanh/sigmoid on ScalarE).
  - Matmul operands are bf16; PSUM fp32; activations/cell state bf16.

Measured on trn2 (512 steps): 1.649 ms (3177 ns/step steady; baseline 1.979 ms).
The period is bound by the serial per-step chain
  h-mul_a 226 (h-mul_b overlaps the a-half matmuls) -> rec-MM(f,i halves)
  -> sigmoid(f|i) 687 -> tanh(g) 363 -> i*g 292 -> c-add 292
  -> tanh(c) 507 (+ ~260 semaphore gaps),
with ScalarE ~64% busy. The h-mul/rec batch-half split relies on SBUF
subtile dependency tracking (verified on HW: rec_f_a starts 53ns after
HMUL_a completes); g,o recurrent MMs stay full-width so their semaphore
release coincides with the b-halves' and the scheduler cannot hoist them
between the f,i halves. Explored and rejected (all lose to the ~260-360ns
fixed per-instruction overheads of ScalarE/VectorE at FD=128-256, or to
head-of-line blocking in the engines' FIFO queues): 2-way batch-split
software pipelining, tanh-as-sigmoid gate folding, tensor_tensor_scan
fusion of the c-update, batch-on-partition transposed layouts, splitting
the tanh(c)/h tail, GPSIMD offload, HAM warm-up/filler matmuls (PE runs
cold at 1.2 GHz; warming it is worth ~240ns/step but fillers block the
recurrent MMs in the FIFO and HAM re-throttles regardless — N=256 and N=64
fillers both measured worse), and batch-half splitting of HMUL+recurrent
MMs (+38ns/step: the scheduler hoists a g-half ahead of the f,i halves,
delaying the f,i sigmoid more than the earlier h-half start saves).
"""

import numpy as np
from contextlib import contextmanager

B, T, IN, H, NCLS = 2048, 512, 2, 64, 9
NCORES = 8
BL = B // NCORES          # 256 batch per core
TC = 16                   # timesteps per x chunk DMA

import os as _os

MM_BF16 = _os.environ.get("MM_BF16", "1") == "1"    # bf16 matmul operands
ACT_BF16 = _os.environ.get("ACT_BF16", "1") == "1"  # bf16 gate activations
C_BF16 = _os.environ.get("C_BF16", "1") == "1"      # bf16 cell state
HSPLIT = _os.environ.get("HSPLIT", "1") == "1"      # batch-half h/rec split
XR = 3  # x rows: x0, x1, ones

LAST_EXEC_NS = None
LAST_TRACE = None

_CACHE = {}


def _np_mmdt():
    if MM_BF16:
        import ml_dtypes
        return ml_dtypes.bfloat16
    return np.float32


def _build(steps=T):
    import concourse.bacc as bacc
    import concourse.tile as tile
    from concourse import mybir
    from contextlib import ExitStack

    f32 = mybir.dt.float32
    mdt = mybir.dt.bfloat16 if MM_BF16 else f32
    adt = mybir.dt.bfloat16 if ACT_BF16 else f32
    cdt = mybir.dt.bfloat16 if C_BF16 else f32
    AF = mybir.ActivationFunctionType
    SIG, TANH = AF.Sigmoid, AF.Tanh

    nc = bacc.Bacc(None, target_bir_lowering=False)

    x_d = nc.dram_tensor("x", [XR, steps * BL], mdt, kind="ExternalInput")
    w1_d = nc.dram_tensor("w1", [2 * H, 4 * H], mdt, kind="ExternalInput")   # [w_ih1;w_hh1].T
    wc_d = nc.dram_tensor("wc", [2 * H, 8 * H], mdt, kind="ExternalInput")   # fused per-gate [128,128]
    w0xb_d = nc.dram_tensor("w0xb", [XR, 8 * H], mdt, kind="ExternalInput")  # x-inject+bias per-gate [3,128]
    bias_d = nc.dram_tensor("bias", [2 * H, 4], f32, kind="ExternalInput")
    wfc_d = nc.dram_tensor("wfc", [2 * H, NCLS], mdt, kind="ExternalInput")  # top half zeros
    bfc_d = nc.dram_tensor("bfc", [NCLS, 1], f32, kind="ExternalInput")
    out_d = nc.dram_tensor("out", [NCLS, BL], f32, kind="ExternalOutput")

    with tile.TileContext(nc) as tc, ExitStack() as ctx:
        consts = ctx.enter_context(tc.tile_pool(name="consts", bufs=1))
        state = ctx.enter_context(tc.tile_pool(name="state", bufs=1))
        xpool = ctx.enter_context(tc.tile_pool(name="xchunk", bufs=2))
        acts = ctx.enter_context(tc.tile_pool(name="acts", bufs=2))
        NWARM = int(_os.environ.get("WARMUP", "0"))
        NFILL = int(_os.environ.get("FILLERS", "0"))
        # 8 PSUM banks: pfi + pgo tags, [128, 512] (1 bank) each, 4-deep
        # rotation -> x-injects for step t+4 can run as soon as step t's
        # activations have read their banks. (3-deep + 1 scratch bank when
        # the HAM warmup/filler experiment is enabled.)
        psum = ctx.enter_context(tc.tile_pool(
            name="psum", bufs=(3 if (NWARM or NFILL) else 4), space="PSUM"))

        def load_const(shape, dt, dram, tag):
            t = consts.tile(shape, dt, tag=tag)
            nc.sync.dma_start(t[:], dram[:, :])
            return t

        # Load order matters: the Sync DMA queue serializes at ~1.4us per
        # transfer, so fetch what the first compute needs (x chunk, w0xb for
        # the x-injects, wc for the recurrent MMs) before the tail consts
        # used only later.
        tc0 = min(TC, steps)
        xch0 = xpool.tile([XR, TC * BL], mdt)
        nc.sync.dma_start(xch0[:, : tc0 * BL], x_d[:, 0 : tc0 * BL])
        w0xb = load_const([XR, 8 * H], mdt, w0xb_d, "w0xb")
        wc = load_const([2 * H, 8 * H], mdt, wc_d, "wc")
        w1 = load_const([2 * H, 4 * H], mdt, w1_d, "w1")
        bias = load_const([2 * H, 4], f32, bias_d, "bias")
        wfc = load_const([2 * H, NCLS], mdt, wfc_d, "wfc")
        bfc = load_const([NCLS, 1], f32, bfc_d, "bfc")

        h_all = state.tile([2 * H, BL], mdt)
        c_all = state.tile([2 * H, BL], cdt)
        nc.vector.memset(h_all[:], 0.0)
        nc.vector.memset(c_all[:], 0.0)

        # gate order in the 4H weight dim (PyTorch): i, f, g, o
        GI, GF, GG, GO = 0, 1, 2, 3
        gsl = lambda j: slice(j * H, (j + 1) * H)
        BW = 512
        # One PSUM bank (512 f32) per gate PAIR: pfi = [f | i], pgo = [g | o],
        # each [128, 512] with the two gates at cols 0:256 / 256:512. The
        # pair's first x-inject uses start=True (clears the whole bank); the
        # second uses start=False, which OVERWRITES its freshly-cleared region
        # (has_written=0) rather than accumulating. Recurrent MMs then
        # accumulate (has_written=1). Contiguous pairs make SIGfi a plain 2D
        # FD=512 read, and 1-bank tiles allow bufs=4 (8 banks total) for
        # deeper x-inject lookahead.
        GLOC = {GF: (0, 0, True), GI: (0, BL, False),
                GG: (1, 0, True), GO: (1, BL, False)}

        xch = None
        cur = None          # psum tiles (pfi, pgo) for the current step
        nxt = None          # psum tiles pre-written with x-inject for step t+1

        # HAM warm-up + per-step fillers (experiment, off by default): a ~3us
        # dense MM burst flips the PE clock gate 1.2 -> 2.4 GHz (measured:
        # spacing 107 -> 56ns after ~25 MMs) and fillers try to keep it warm.
        # Measured net-NEGATIVE: the fillers head-of-line-block the recurrent
        # MMs in the PE FIFO (+120ns/step) and HAM re-throttles anyway.
        scratch = None
        if NWARM or NFILL:
            scratch = psum.tile([2 * H, BW], f32, tag="scratch", bufs=1)
        for _ in range(NWARM):
            nc.tensor.matmul(scratch[:, 0:128], lhsT=wc[:, 0:128],
                             rhs=wc[:, 0:128], start=True, stop=True,
                             skip_group_check=True)

        def xinj(t, xt):
            """Allocate psum tiles for step t and run the 4 x-inject MMs."""
            pfi = psum.tile([2 * H, BW], f32, tag="pfi")
            pgo = psum.tile([2 * H, BW], f32, tag="pgo")
            tiles = (pfi, pgo)
            for j in (GF, GI, GG, GO):
                ti, co, st = GLOC[j]
                nc.tensor.matmul(
                    tiles[ti][:, co : co + BL],
                    lhsT=w0xb[:, 2 * H * j : 2 * H * (j + 1)],
                    rhs=xt, start=st, stop=False, skip_group_check=True,
                )
            return tiles

        def xt_of(t):
            return xch[:, (t % TC) * BL : (t % TC + 1) * BL]

        for it in range(steps + 1):
            do0 = it < steps

            if it == 0:
                xch = xch0
                nxt = xinj(0, xt_of(0))

            cur, nxt = nxt, None

            if do0:
                # current step's gate matmuls first need h(t); meanwhile the
                # next step's x-injects (below, issued first in FIFO) run.
                if it + 1 < steps:
                    if (it + 1) % TC == 0:
                        tc_nx = min(TC, steps - (it + 1))
                        xch = xpool.tile([XR, TC * BL], mdt)
                        nc.sync.dma_start(
                            xch[:, : tc_nx * BL],
                            x_d[:, (it + 1) * BL : (it + 1 + tc_nx) * BL],
                        )
                    nxt = xinj(it + 1, xt_of(it + 1))
                # recurrent + layer1-inject: one K=128 MM per gate; the f,i
                # MMs are split into batch halves whose rhs is a column-slice
                # of h_all, so (if SBUF subtile deps hold) the a-half starts
                # after only HMUL_a. g,o stay full-width: their release then
                # coincides with the b-halves', so the scheduler cannot hoist
                # them between the f,i halves (the v8 failure mode).
                if HSPLIT:
                    for j, cs in ((GF, slice(0, BL // 2)),
                                  (GI, slice(0, BL // 2)),
                                  (GF, slice(BL // 2, BL)),
                                  (GI, slice(BL // 2, BL))):
                        ti, co, _ = GLOC[j]
                        nc.tensor.matmul(
                            cur[ti][:, co + cs.start : co + cs.stop],
                            lhsT=wc[:, 2 * H * j : 2 * H * (j + 1)],
                            rhs=h_all[:, cs], start=False, stop=True,
                            skip_group_check=True,
                        )
                    rec_gates = (GG, GO)
                else:
                    rec_gates = (GF, GI, GG, GO)
                for j in rec_gates:
                    ti, co, _ = GLOC[j]
                    nc.tensor.matmul(
                        cur[ti][:, co : co + BL],
                        lhsT=wc[:, 2 * H * j : 2 * H * (j + 1)],
                        rhs=h_all[:, :], start=False, stop=True,
                        skip_group_check=True,
                    )
                lo, hi = 0, (2 * H if it >= 1 else H)
            else:
                # final iteration: layer1 only @ t = steps-1
                pfi = psum.tile([2 * H, BW], f32, tag="pfi")
                pgo = psum.tile([2 * H, BW], f32, tag="pgo")
                cur = (pfi, pgo)
                for j in (GF, GI, GG, GO):
                    ti, co, st = GLOC[j]
                    nc.tensor.matmul(
                        cur[ti][H : 2 * H, co : co + BL],
                        lhsT=w1[:, gsl(j)], rhs=h_all[:, :],
                        start=st, stop=True, skip_group_check=True,
                    )
                lo, hi = H, 2 * H

            sl = slice(lo, hi)
            bias_kw = {}
            if not do0:
                # biases normally ride the x-inject ones-row; the final
                # L1-only step has no x-inject, so use the ACT bias operand.
                bias_kw = {GF: dict(bias=bias[sl, GF : GF + 1]),
                           GI: dict(bias=bias[sl, GI : GI + 1]),
                           GG: dict(bias=bias[sl, GG : GG + 1]),
                           GO: dict(bias=bias[sl, GO : GO + 1])}

            s_fi = acts.tile([2 * H, 2 * BL], adt, tag="sfi")
            if do0:
                nc.scalar.activation(s_fi[sl, :], cur[0][sl, :], SIG)
                # HAM fillers: N=32 MMs run at the NX dispatch floor (~35ns
                # at BOTH clock states), so the cold->warm transition doesn't
                # change the drain time. lhsT reads s_fi, so they cannot run
                # before this step's sigmoid - the earlier-released x-injects
                # get scheduled ahead of them and the fillers drain during
                # the activation/vector tail, before rec(t+1) is ready.
                for _ in range(NFILL):
                    nc.tensor.matmul(scratch[0:32, 0:32],
                                     lhsT=s_fi[:, 0:32], rhs=wc[:, 0:32],
                                     start=True, stop=True,
                                     skip_group_check=True)
            else:
                nc.scalar.activation(s_fi[sl, 0:BL], cur[0][sl, 0:BL], SIG,
                                     **bias_kw[GF])
                nc.scalar.activation(s_fi[sl, BL : 2 * BL], cur[0][sl, BL : 2 * BL],
                                     SIG, **bias_kw[GI])
            s_g = acts.tile([2 * H, BL], adt, tag="sg")
            nc.scalar.activation(s_g[sl, :], cur[1][sl, 0:BL], TANH,
                                 **(bias_kw.get(GG, {}) if not do0 else {}))
            s_o = acts.tile([2 * H, BL], adt, tag="so")
            nc.scalar.activation(s_o[sl, :], cur[1][sl, BL : 2 * BL], SIG,
                                 **(bias_kw.get(GO, {}) if not do0 else {}))
            f_ap, i_ap = s_fi[:, 0:BL], s_fi[:, BL : 2 * BL]

            t_fc = acts.tile([2 * H, BL], cdt, tag="tfc")
            nc.vector.tensor_mul(t_fc[sl, :], f_ap[sl, :], c_all[sl, :])
            t_ig = acts.tile([2 * H, BL], cdt, tag="tig")
            nc.vector.tensor_mul(t_ig[sl, :], i_ap[sl, :], s_g[sl, :])
            nc.vector.tensor_add(c_all[sl, :], t_fc[sl, :], t_ig[sl, :])
            s_tc = acts.tile([2 * H, BL], adt, tag="stc")
            nc.scalar.activation(s_tc[sl, :], c_all[sl, :], TANH)
            if HSPLIT:
                HB = BL // 2
                nc.vector.tensor_mul(h_all[sl, 0:HB], s_o[sl, 0:HB],
                                     s_tc[sl, 0:HB])
                nc.vector.tensor_mul(h_all[sl, HB:BL], s_o[sl, HB:BL],
                                     s_tc[sl, HB:BL])
            else:
                nc.vector.tensor_mul(h_all[sl, :], s_o[sl, :], s_tc[sl, :])

        # FC head on h2(T-1) = h_all[64:128]; wfc is zero-padded on rows 0:64
        p_fc = psum.tile([2 * H, BW], f32, tag="pfi")
        nc.tensor.matmul(p_fc[0:NCLS, 0:BL], lhsT=wfc[:, :], rhs=h_all[:, :],
                         start=True, stop=True)
        o_sb = acts.tile([2 * H, BL], f32, tag="osb")
        nc.vector.tensor_scalar_add(o_sb[0:NCLS, :], p_fc[0:NCLS, 0:BL], bfc[:, 0:1])
        nc.sync.dma_start(out_d[:, :], o_sb[0:NCLS, :])

    nc.finalize()
    return nc


def _prep_weights(w_ih0, w_hh0, b_ih0, b_hh0, w_ih1, w_hh1, b_ih1, b_hh1, w_fc, b_fc):
    f = np.float32
    mdt = _np_mmdt()
    w1 = np.ascontiguousarray(
        np.concatenate([np.asarray(w_ih1), np.asarray(w_hh1)], 1).T
    ).astype(mdt)                                                        # [128, 256]
    # fused per-gate [K=128, M=128] blocks: cols 0:64 -> layer0 gate (zeros on
    # h2 rows), cols 64:128 -> layer1 gate ([w_ih1; w_hh1])
    wcf = np.zeros((2 * H, 8 * H), dtype=np.float32)
    w1f = np.concatenate([np.asarray(w_ih1), np.asarray(w_hh1)], 1)  # [256, 128]
    for g in range(4):
        wcf[0:H, 2 * H * g : 2 * H * g + H] = np.asarray(w_hh0)[g * H:(g + 1) * H, :].T
        wcf[:, 2 * H * g + H : 2 * H * (g + 1)] = w1f[g * H:(g + 1) * H, :].T
    wc = np.ascontiguousarray(wcf).astype(mdt)
    b0v = (np.asarray(b_ih0) + np.asarray(b_hh0)).astype(np.float32)
    b1v = (np.asarray(b_ih1) + np.asarray(b_hh1)).astype(np.float32)
    w0xbf = np.zeros((XR, 8 * H), dtype=np.float32)
    for g in range(4):
        w0xbf[0:IN, 2 * H * g : 2 * H * g + H] = \
            np.asarray(w_ih0)[g * H:(g + 1) * H, :].T
        w0xbf[IN, 2 * H * g : 2 * H * g + H] = b0v[g * H:(g + 1) * H]
        w0xbf[IN, 2 * H * g + H : 2 * H * (g + 1)] = b1v[g * H:(g + 1) * H]
    w0xb = np.ascontiguousarray(w0xbf).astype(mdt)
    b0 = (np.asarray(b_ih0) + np.asarray(b_hh0)).astype(f).reshape(4, H)
    b1 = (np.asarray(b_ih1) + np.asarray(b_hh1)).astype(f).reshape(4, H)
    bias = np.ascontiguousarray(np.concatenate([b0.T, b1.T], axis=0), dtype=f)
    wfc = np.zeros((2 * H, NCLS), dtype=f)
    wfc[H:, :] = np.asarray(w_fc).T
    wfc = wfc.astype(mdt)
    bfc = np.ascontiguousarray(np.asarray(b_fc).reshape(NCLS, 1), dtype=f)
    return dict(w1=w1, wc=wc, w0xb=w0xb, bias=bias, wfc=wfc, bfc=bfc)


def _prep_x(x, steps=T):
    mdt = _np_mmdt()
    x = np.asarray(x, dtype=np.float32)
    per_core = []
    for c in range(NCORES):
        xc = x[c * BL : (c + 1) * BL, :steps, :]          # [BL, steps, IN]
        xc = xc.transpose(2, 1, 0).reshape(IN, steps * BL)  # [IN, steps*BL]
        xa = np.ones((XR, steps * BL), dtype=np.float32)
        xa[0:IN] = xc
        per_core.append(np.ascontiguousarray(xa).astype(mdt))
    return per_core


@contextmanager
def _fast_compile():
    """Disable walrus birsim (compile-time BIR simulation): it costs ~7s per
    LSTM step (~1h for T=512) and only re-verifies what CoreSim already
    checked. NEFF output is identical."""
    import concourse.bass_utils as bu

    orig = bu.run_command

    def patched(argv, **kw):
        argv = [
            a.replace("--enable-birsim=true", "--enable-birsim=false")
            if isinstance(a, str) else a
            for a in argv
        ]
        return orig(argv, **kw)

    bu.run_command = patched
    try:
        yield
    finally:
        bu.run_command = orig


def kernel(x, w_ih0, w_hh0, b_ih0, b_hh0, w_ih1, w_hh1, b_ih1, b_hh1,
           w_fc, b_fc, _steps=T, _trace=False):
    global LAST_EXEC_NS, LAST_TRACE
    from concourse.bass_utils import run_bass_kernel_spmd

    key = ("nc", _steps)
    if key not in _CACHE:
        _CACHE[key] = _build(steps=_steps)
    nc = _CACHE[key]

    wmap = _prep_weights(w_ih0, w_hh0, b_ih0, b_hh0,
                         w_ih1, w_hh1, b_ih1, b_hh1, w_fc, b_fc)
    xs = _prep_x(x, _steps)
    in_maps = [{"x": xs[c], **wmap} for c in range(NCORES)]

    with _fast_compile():
        res = run_bass_kernel_spmd(nc, in_maps, core_ids=list(range(NCORES)),
                                   trace=_trace)
    LAST_EXEC_NS = res.exec_time_ns
    LAST_TRACE = res.instructions_and_trace
    out = np.concatenate([r["out"].T for r in res.results], axis=0)  # [B, 9]
    return out.astype(np.float32)


# revision 40
# speedup vs baseline: 1.0412x; 1.0087x over previous
"""Trainium2 Bass kernel for a 2-layer LSTM (B=2048, T=512, IN=2, H=64) + FC head.

Data-parallel over 8 NeuronCores: batch 2048 -> 256 per core, weights replicated.

On-chip layout (per core, B_local = 256):
  - Hidden/cell state of BOTH layers packed in one [128, 256] SBUF tile:
    rows 0:64 = layer0 h (or c), rows 64:128 = layer1 h (or c); free dim = batch.
  - Software pipeline: iteration i computes layer0 @ t=i and layer1 @ t=i-1.
  - PSUM: one bank (512 f32) per gate PAIR, two tiles per step: pfi = [f|i],
    pgo = [g|o], 4-deep rotation (8 banks total). Per-pair tiles make the
    Tile dependency tracker release the f,i sigmoid after only the f,i
    recurrent matmuls; the 4-deep rotation lets x-inject matmuls for step
    t+4 run during step t's activation tail (keeps the PE dense and off the
    critical path).
  - Per step the PE runs [x-inject(t+1) K=3 MMs x4] then [recurrent K=128
    MMs x4]; x-injects are emitted first (no h(t) dependency).
  - Biases ride the ones-row of the x-inject matmul (w0xb).
  - VectorE: c = f*c + i*g ; h = o * tanh(c)  (tanh/sigmoid on ScalarE).
  - Matmul operands are bf16; PSUM fp32; activations/cell state bf16.

Measured on trn2 (512 steps): 1.649 ms (3177 ns/step steady; baseline 1.979 ms).
The period is bound by the serial per-step chain
  h-mul_a 226 (h-mul_b overlaps the a-half matmuls) -> rec-MM(f,i halves)
  -> sigmoid(f|i) 687 -> tanh(g) 363 -> i*g 292 -> c-add 292
  -> tanh(c) 507 (+ ~260 semaphore gaps),
with ScalarE ~64% busy. The h-mul/rec batch-half split relies on SBUF
subtile dependency tracking (verified on HW: rec_f_a starts 53ns after
HMUL_a completes); g,o recurrent MMs stay full-width so their semaphore
release coincides with the b-halves' and the scheduler cannot hoist them
between the f,i halves. Explored and rejected (all lose to the ~260-360ns
fixed per-instruction overheads of ScalarE/VectorE at FD=128-256, or to
head-of-line blocking in the engines' FIFO queues): 2-way batch-split
software pipelining, tanh-as-sigmoid gate folding, tensor_tensor_scan
fusion of the c-update, batch-on-partition transposed layouts, splitting
the tanh(c)/h tail, GPSIMD offload, and HAM warm-up/filler matmuls (PE
runs cold at 1.2 GHz; warming it is worth ~240ns/step, but fillers at
N=256, N=64, and even N=32 at the clock-invariant ~27ns NX dispatch floor
with a SIGfi-gating dependency ALL measured worse - the scheduler's static
FIFO placement always lands the filler block across a chain handoff, and
HAM re-throttles at anything under ~85% PE busy regardless).
"""

import numpy as np
from contextlib import contextmanager

B, T, IN, H, NCLS = 2048, 512, 2, 64, 9
NCORES = 8
BL = B // NCORES          # 256 batch per core
TC = 16                   # timesteps per x chunk DMA

import os as _os

MM_BF16 = _os.environ.get("MM_BF16", "1") == "1"    # bf16 matmul operands
ACT_BF16 = _os.environ.get("ACT_BF16", "1") == "1"  # bf16 gate activations
C_BF16 = _os.environ.get("C_BF16", "1") == "1"      # bf16 cell state
HSPLIT = _os.environ.get("HSPLIT", "1") == "1"      # batch-half h/rec split
TSPLIT = _os.environ.get("TSPLIT", "0") == "1"      # batch-half tanh(c) split
XR = 3  # x rows: x0, x1, ones

LAST_EXEC_NS = None
LAST_TRACE = None

_CACHE = {}


def _np_mmdt():
    if MM_BF16:
        import ml_dtypes
        return ml_dtypes.bfloat16
    return np.float32


def _build(steps=T):
    import concourse.bacc as bacc
    import concourse.tile as tile
    from concourse import mybir
    from contextlib import ExitStack

    f32 = mybir.dt.float32
    mdt = mybir.dt.bfloat16 if MM_BF16 else f32
    adt = mybir.dt.bfloat16 if ACT_BF16 else f32
    cdt = mybir.dt.bfloat16 if C_BF16 else f32
    AF = mybir.ActivationFunctionType
    SIG, TANH = AF.Sigmoid, AF.Tanh

    nc = bacc.Bacc(None, target_bir_lowering=False)

    x_d = nc.dram_tensor("x", [XR, steps * BL], mdt, kind="ExternalInput")
    w1_d = nc.dram_tensor("w1", [2 * H, 4 * H], mdt, kind="ExternalInput")   # [w_ih1;w_hh1].T
    wc_d = nc.dram_tensor("wc", [2 * H, 8 * H], mdt, kind="ExternalInput")   # fused per-gate [128,128]
    w0xb_d = nc.dram_tensor("w0xb", [XR, 8 * H], mdt, kind="ExternalInput")  # x-inject+bias per-gate [3,128]
    bias_d = nc.dram_tensor("bias", [2 * H, 4], f32, kind="ExternalInput")
    wfc_d = nc.dram_tensor("wfc", [2 * H, NCLS], mdt, kind="ExternalInput")  # top half zeros
    bfc_d = nc.dram_tensor("bfc", [NCLS, 1], f32, kind="ExternalInput")
    out_d = nc.dram_tensor("out", [NCLS, BL], f32, kind="ExternalOutput")

    with tile.TileContext(nc) as tc, ExitStack() as ctx:
        consts = ctx.enter_context(tc.tile_pool(name="consts", bufs=1))
        state = ctx.enter_context(tc.tile_pool(name="state", bufs=1))
        xpool = ctx.enter_context(tc.tile_pool(name="xchunk", bufs=2))
        acts = ctx.enter_context(tc.tile_pool(name="acts", bufs=2))
        NWARM = int(_os.environ.get("WARMUP", "0"))
        NFILL = int(_os.environ.get("FILLERS", "0"))
        # 8 PSUM banks: pfi + pgo tags, [128, 512] (1 bank) each, 4-deep
        # rotation -> x-injects for step t+4 can run as soon as step t's
        # activations have read their banks. (3-deep + 1 scratch bank when
        # the HAM warmup/filler experiment is enabled.)
        psum = ctx.enter_context(tc.tile_pool(
            name="psum", bufs=(3 if (NWARM or NFILL) else 4), space="PSUM"))

        def load_const(shape, dt, dram, tag):
            t = consts.tile(shape, dt, tag=tag)
            nc.sync.dma_start(t[:], dram[:, :])
            return t

        # Load order matters: the Sync DMA queue serializes at ~1.4us per
        # transfer, so fetch what the first compute needs (x chunk, w0xb for
        # the x-injects, wc for the recurrent MMs) before the tail consts
        # used only later.
        tc0 = min(TC, steps)
        xch0 = xpool.tile([XR, TC * BL], mdt)
        nc.sync.dma_start(xch0[:, : tc0 * BL], x_d[:, 0 : tc0 * BL])
        w0xb = load_const([XR, 8 * H], mdt, w0xb_d, "w0xb")
        wc = load_const([2 * H, 8 * H], mdt, wc_d, "wc")
        w1 = load_const([2 * H, 4 * H], mdt, w1_d, "w1")
        bias = load_const([2 * H, 4], f32, bias_d, "bias")
        wfc = load_const([2 * H, NCLS], mdt, wfc_d, "wfc")
        bfc = load_const([NCLS, 1], f32, bfc_d, "bfc")

        h_all = state.tile([2 * H, BL], mdt)
        c_all = state.tile([2 * H, BL], cdt)
        nc.vector.memset(h_all[:], 0.0)
        nc.vector.memset(c_all[:], 0.0)

        # gate order in the 4H weight dim (PyTorch): i, f, g, o
        GI, GF, GG, GO = 0, 1, 2, 3
        gsl = lambda j: slice(j * H, (j + 1) * H)
        BW = 512
        # One PSUM bank (512 f32) per gate PAIR: pfi = [f | i], pgo = [g | o],
        # each [128, 512] with the two gates at cols 0:256 / 256:512. The
        # pair's first x-inject uses start=True (clears the whole bank); the
        # second uses start=False, which OVERWRITES its freshly-cleared region
        # (has_written=0) rather than accumulating. Recurrent MMs then
        # accumulate (has_written=1). Contiguous pairs make SIGfi a plain 2D
        # FD=512 read, and 1-bank tiles allow bufs=4 (8 banks total) for
        # deeper x-inject lookahead.
        GLOC = {GF: (0, 0, True), GI: (0, BL, False),
                GG: (1, 0, True), GO: (1, BL, False)}

        xch = None
        cur = None          # psum tiles (pfi, pgo) for the current step
        nxt = None          # psum tiles pre-written with x-inject for step t+1

        # HAM warm-up + per-step fillers (experiment, off by default): a ~3us
        # dense MM burst flips the PE clock gate 1.2 -> 2.4 GHz (measured:
        # spacing 107 -> 56ns after ~25 MMs) and fillers try to keep it warm.
        # Measured net-NEGATIVE: the fillers head-of-line-block the recurrent
        # MMs in the PE FIFO (+120ns/step) and HAM re-throttles anyway.
        scratch = None
        if NWARM or NFILL:
            scratch = psum.tile([2 * H, BW], f32, tag="scratch", bufs=1)
        for _ in range(NWARM):
            nc.tensor.matmul(scratch[:, 0:128], lhsT=wc[:, 0:128],
                             rhs=wc[:, 0:128], start=True, stop=True,
                             skip_group_check=True)

        def xinj(t, xt):
            """Allocate psum tiles for step t and run the 4 x-inject MMs."""
            pfi = psum.tile([2 * H, BW], f32, tag="pfi")
            pgo = psum.tile([2 * H, BW], f32, tag="pgo")
            tiles = (pfi, pgo)
            for j in (GF, GI, GG, GO):
                ti, co, st = GLOC[j]
                nc.tensor.matmul(
                    tiles[ti][:, co : co + BL],
                    lhsT=w0xb[:, 2 * H * j : 2 * H * (j + 1)],
                    rhs=xt, start=st, stop=False, skip_group_check=True,
                )
            return tiles

        def xt_of(t):
            return xch[:, (t % TC) * BL : (t % TC + 1) * BL]

        for it in range(steps + 1):
            do0 = it < steps

            if it == 0:
                xch = xch0
                nxt = xinj(0, xt_of(0))

            cur, nxt = nxt, None

            if do0:
                # current step's gate matmuls first need h(t); meanwhile the
                # next step's x-injects (below, issued first in FIFO) run.
                if it + 1 < steps:
                    if (it + 1) % TC == 0:
                        tc_nx = min(TC, steps - (it + 1))
                        xch = xpool.tile([XR, TC * BL], mdt)
                        nc.sync.dma_start(
                            xch[:, : tc_nx * BL],
                            x_d[:, (it + 1) * BL : (it + 1 + tc_nx) * BL],
                        )
                    nxt = xinj(it + 1, xt_of(it + 1))
                # recurrent + layer1-inject: one K=128 MM per gate; the f,i
                # MMs are split into batch halves whose rhs is a column-slice
                # of h_all, so (if SBUF subtile deps hold) the a-half starts
                # after only HMUL_a. g,o stay full-width: their release then
                # coincides with the b-halves', so the scheduler cannot hoist
                # them between the f,i halves (the v8 failure mode).
                if HSPLIT:
                    for j, cs in ((GF, slice(0, BL // 2)),
                                  (GI, slice(0, BL // 2)),
                                  (GF, slice(BL // 2, BL)),
                                  (GI, slice(BL // 2, BL))):
                        ti, co, _ = GLOC[j]
                        nc.tensor.matmul(
                            cur[ti][:, co + cs.start : co + cs.stop],
                            lhsT=wc[:, 2 * H * j : 2 * H * (j + 1)],
                            rhs=h_all[:, cs], start=False, stop=True,
                            skip_group_check=True,
                        )
                    rec_gates = (GG, GO)
                else:
                    rec_gates = (GF, GI, GG, GO)
                for j in rec_gates:
                    ti, co, _ = GLOC[j]
                    nc.tensor.matmul(
                        cur[ti][:, co : co + BL],
                        lhsT=wc[:, 2 * H * j : 2 * H * (j + 1)],
                        rhs=h_all[:, :], start=False, stop=True,
                        skip_group_check=True,
                    )
                lo, hi = 0, (2 * H if it >= 1 else H)
            else:
                # final iteration: layer1 only @ t = steps-1
                pfi = psum.tile([2 * H, BW], f32, tag="pfi")
                pgo = psum.tile([2 * H, BW], f32, tag="pgo")
                cur = (pfi, pgo)
                for j in (GF, GI, GG, GO):
                    ti, co, st = GLOC[j]
                    nc.tensor.matmul(
                        cur[ti][H : 2 * H, co : co + BL],
                        lhsT=w1[:, gsl(j)], rhs=h_all[:, :],
                        start=st, stop=True, skip_group_check=True,
                    )
                lo, hi = H, 2 * H

            sl = slice(lo, hi)
            bias_kw = {}
            if not do0:
                # biases normally ride the x-inject ones-row; the final
                # L1-only step has no x-inject, so use the ACT bias operand.
                bias_kw = {GF: dict(bias=bias[sl, GF : GF + 1]),
                           GI: dict(bias=bias[sl, GI : GI + 1]),
                           GG: dict(bias=bias[sl, GG : GG + 1]),
                           GO: dict(bias=bias[sl, GO : GO + 1])}

            s_fi = acts.tile([2 * H, 2 * BL], adt, tag="sfi")
            if do0:
                nc.scalar.activation(s_fi[sl, :], cur[0][sl, :], SIG)
                # HAM fillers: N=32 MMs run at the NX dispatch floor (~35ns
                # at BOTH clock states), so the cold->warm transition doesn't
                # change the drain time. lhsT reads s_fi, so they cannot run
                # before this step's sigmoid - the earlier-released x-injects
                # get scheduled ahead of them and the fillers drain during
                # the activation/vector tail, before rec(t+1) is ready.
                for _ in range(NFILL):
                    nc.tensor.matmul(scratch[0:32, 0:32],
                                     lhsT=s_fi[:, 0:32], rhs=wc[:, 0:32],
                                     start=True, stop=True,
                                     skip_group_check=True)
            else:
                nc.scalar.activation(s_fi[sl, 0:BL], cur[0][sl, 0:BL], SIG,
                                     **bias_kw[GF])
                nc.scalar.activation(s_fi[sl, BL : 2 * BL], cur[0][sl, BL : 2 * BL],
                                     SIG, **bias_kw[GI])
            s_g = acts.tile([2 * H, BL], adt, tag="sg")
            nc.scalar.activation(s_g[sl, :], cur[1][sl, 0:BL], TANH,
                                 **(bias_kw.get(GG, {}) if not do0 else {}))
            s_o = acts.tile([2 * H, BL], adt, tag="so")
            nc.scalar.activation(s_o[sl, :], cur[1][sl, BL : 2 * BL], SIG,
                                 **(bias_kw.get(GO, {}) if not do0 else {}))
            f_ap, i_ap = s_fi[:, 0:BL], s_fi[:, BL : 2 * BL]

            t_fc = acts.tile([2 * H, BL], cdt, tag="tfc")
            nc.vector.tensor_mul(t_fc[sl, :], f_ap[sl, :], c_all[sl, :])
            t_ig = acts.tile([2 * H, BL], cdt, tag="tig")
            nc.vector.tensor_mul(t_ig[sl, :], i_ap[sl, :], s_g[sl, :])
            nc.vector.tensor_add(c_all[sl, :], t_fc[sl, :], t_ig[sl, :])
            s_tc = acts.tile([2 * H, BL], adt, tag="stc")
            if TSPLIT and do0:
                # tanh(c) in batch halves: both release at the c-add, so the
                # b-half queues on ScalarE with its dispatch hidden; HMUL_a
                # then starts ~100ns earlier via the SBUF-subtile release.
                nc.scalar.activation(s_tc[sl, 0 : BL // 2],
                                     c_all[sl, 0 : BL // 2], TANH)
                nc.scalar.activation(s_tc[sl, BL // 2 : BL],
                                     c_all[sl, BL // 2 : BL], TANH)
            else:
                nc.scalar.activation(s_tc[sl, :], c_all[sl, :], TANH)
            if HSPLIT:
                HB = BL // 2
                nc.vector.tensor_mul(h_all[sl, 0:HB], s_o[sl, 0:HB],
                                     s_tc[sl, 0:HB])
                nc.vector.tensor_mul(h_all[sl, HB:BL], s_o[sl, HB:BL],
                                     s_tc[sl, HB:BL])
            else:
                nc.vector.tensor_mul(h_all[sl, :], s_o[sl, :], s_tc[sl, :])

        # FC head on h2(T-1) = h_all[64:128]; wfc is zero-padded on rows 0:64
        p_fc = psum.tile([2 * H, BW], f32, tag="pfi")
        nc.tensor.matmul(p_fc[0:NCLS, 0:BL], lhsT=wfc[:, :], rhs=h_all[:, :],
                         start=True, stop=True)
        o_sb = acts.tile([2 * H, BL], f32, tag="osb")
        nc.vector.tensor_scalar_add(o_sb[0:NCLS, :], p_fc[0:NCLS, 0:BL], bfc[:, 0:1])
        nc.sync.dma_start(out_d[:, :], o_sb[0:NCLS, :])

    nc.finalize()
    return nc


def _prep_weights(w_ih0, w_hh0, b_ih0, b_hh0, w_ih1, w_hh1, b_ih1, b_hh1, w_fc, b_fc):
    f = np.float32
    mdt = _np_mmdt()
    w1 = np.ascontiguousarray(
        np.concatenate([np.asarray(w_ih1), np.asarray(w_hh1)], 1).T
    ).astype(mdt)                                                        # [128, 256]
    # fused per-gate [K=128, M=128] blocks: cols 0:64 -> layer0 gate (zeros on
    # h2 rows), cols 64:128 -> layer1 gate ([w_ih1; w_hh1])
    wcf = np.zeros((2 * H, 8 * H), dtype=np.float32)
    w1f = np.concatenate([np.asarray(w_ih1), np.asarray(w_hh1)], 1)  # [256, 128]
    for g in range(4):
        wcf[0:H, 2 * H * g : 2 * H * g + H] = np.asarray(w_hh0)[g * H:(g + 1) * H, :].T
        wcf[:, 2 * H * g + H : 2 * H * (g + 1)] = w1f[g * H:(g + 1) * H, :].T
    wc = np.ascontiguousarray(wcf).astype(mdt)
    b0v = (np.asarray(b_ih0) + np.asarray(b_hh0)).astype(np.float32)
    b1v = (np.asarray(b_ih1) + np.asarray(b_hh1)).astype(np.float32)
    w0xbf = np.zeros((XR, 8 * H), dtype=np.float32)
    for g in range(4):
        w0xbf[0:IN, 2 * H * g : 2 * H * g + H] = \
            np.asarray(w_ih0)[g * H:(g + 1) * H, :].T
        w0xbf[IN, 2 * H * g : 2 * H * g + H] = b0v[g * H:(g + 1) * H]
        w0xbf[IN, 2 * H * g + H : 2 * H * (g + 1)] = b1v[g * H:(g + 1) * H]
    w0xb = np.ascontiguousarray(w0xbf).astype(mdt)
    b0 = (np.asarray(b_ih0) + np.asarray(b_hh0)).astype(f).reshape(4, H)
    b1 = (np.asarray(b_ih1) + np.asarray(b_hh1)).astype(f).reshape(4, H)
    bias = np.ascontiguousarray(np.concatenate([b0.T, b1.T], axis=0), dtype=f)
    wfc = np.zeros((2 * H, NCLS), dtype=f)
    wfc[H:, :] = np.asarray(w_fc).T
    wfc = wfc.astype(mdt)
    bfc = np.ascontiguousarray(np.asarray(b_fc).reshape(NCLS, 1), dtype=f)
    return dict(w1=w1, wc=wc, w0xb=w0xb, bias=bias, wfc=wfc, bfc=bfc)


def _prep_x(x, steps=T):
    mdt = _np_mmdt()
    x = np.asarray(x, dtype=np.float32)
    per_core = []
    for c in range(NCORES):
        xc = x[c * BL : (c + 1) * BL, :steps, :]          # [BL, steps, IN]
        xc = xc.transpose(2, 1, 0).reshape(IN, steps * BL)  # [IN, steps*BL]
        xa = np.ones((XR, steps * BL), dtype=np.float32)
        xa[0:IN] = xc
        per_core.append(np.ascontiguousarray(xa).astype(mdt))
    return per_core


@contextmanager
def _fast_compile():
    """Disable walrus birsim (compile-time BIR simulation): it costs ~7s per
    LSTM step (~1h for T=512) and only re-verifies what CoreSim already
    checked. NEFF output is identical."""
    import concourse.bass_utils as bu

    orig = bu.run_command

    def patched(argv, **kw):
        argv = [
            a.replace("--enable-birsim=true", "--enable-birsim=false")
            if isinstance(a, str) else a
            for a in argv
        ]
        return orig(argv, **kw)

    bu.run_command = patched
    try:
        yield
    finally:
        bu.run_command = orig


def kernel(x, w_ih0, w_hh0, b_ih0, b_hh0, w_ih1, w_hh1, b_ih1, b_hh1,
           w_fc, b_fc, _steps=T, _trace=False):
    global LAST_EXEC_NS, LAST_TRACE
    from concourse.bass_utils import run_bass_kernel_spmd

    key = ("nc", _steps)
    if key not in _CACHE:
        _CACHE[key] = _build(steps=_steps)
    nc = _CACHE[key]

    wmap = _prep_weights(w_ih0, w_hh0, b_ih0, b_hh0,
                         w_ih1, w_hh1, b_ih1, b_hh1, w_fc, b_fc)
    xs = _prep_x(x, _steps)
    in_maps = [{"x": xs[c], **wmap} for c in range(NCORES)]

    with _fast_compile():
        res = run_bass_kernel_spmd(nc, in_maps, core_ids=list(range(NCORES)),
                                   trace=_trace)
    LAST_EXEC_NS = res.exec_time_ns
    LAST_TRACE = res.instructions_and_trace
    out = np.concatenate([r["out"].T for r in res.results], axis=0)  # [B, 9]
    return out.astype(np.float32)
